# revision 29
# baseline (speedup 1.0000x reference)
"""Trainium2 Bass kernel for the 2-hop GNN (GCN + SAGE + BatchNorm).

Strategy (8 NeuronCores, SPMD, destination sharding):
  - Core k owns output rows [k*12500, (k+1)*12500); padded to 12544 = 392
    aggregation windows of 32 destinations (output staged per 128).
  - Host prep is pure indexing / integer work: assemble userF by embedding
    lookups+concat (gathers, no arithmetic), bincount degrees/counts, sort
    edges by destination window, and lay the per-edge source rows out as a
    dense window-major stream per core (the per-input edge list is known at
    compile time, so the random-access gather is baked into the stream
    layout; no SWDGE descriptor generation on device).  All FP arithmetic
    runs on device.
  - Device: sequential-stream the edge rows (big contiguous HWDGE DMAs on
    two queues), build S[e,d] = (col[e]==d) * coef[e] in 24-block batches:
    one-hot is_equal on DVE, coef broadcast-mult on GpSimd.  Segment-sum
    via one-hot matmuls accumulating per 32-dest window into 4-window PSUM
    tiles; one PSUM->SBUF copy per 128-dest group on the Scalar engine.
    coef folds dis[row]*dis[col] (resp. 1/cnt[dst]) computed on device
    from uploaded integer degree values.
  - Final: per 512-dest tile, 3 bf16 matmuls per hop + leaky relu
    (relu-pair trick: Relu on ACT, hop-sums on GpSimd, fused combine on
    DVE); final tiles interleave with the streams.  BN stats AllReduced
    once right after the last tile (last GpSimd queue entry, so nothing
    stalls behind the collective); output staged per 8 windows and written
    contiguous bf16 (host relayouts to [N, H] f32).
"""

import numpy as np
import ml_dtypes

import concourse.bass as bass
import concourse.bacc as bacc
import concourse.tile as tile
import concourse.mybir as mybir
from concourse import bass_utils

F32 = mybir.dt.float32
BF16 = mybir.dt.bfloat16

U1 = 50000
U2 = 50000
U = 100000
C = 200000
E = 1000000
ED = 85
DC = 64
H = 128
NH = 2

NCORES = 8
L = 12500
WIN = 32               # aggregation window (dests per one-hot matmul)
NW = 392               # LP / WIN
GRP = 128 // WIN       # windows per PSUM tile / agg copy
LP = NW * WIN          # 12544
OW = 128               # output-stage window
NOW = 98               # LP / OW
CHUNK = 32             # stream blocks per DMA / S-build batch
FTILE = 512
NT = (LP + FTILE - 1) // FTILE   # 25
SPLIT_T = NT           # single end-of-stream stats AllReduce
BF = ml_dtypes.bfloat16


def _bucket_stream(row, col, src_bf, F, aux):
    """Sort edges by (dest core, dest window); emit per-core dense streams.

    Returns per-core arrays {stream, colw, aux...} plus shared block meta.
    The block structure (bmat/blockbase) is shared across cores (SPMD
    program), padded to the worst core per window.
    """
    ne = len(row)
    shard = col // L
    lc = col % L
    w = lc // WIN
    cw = (lc % WIN).astype(np.float32)
    bid = shard * NW + w
    counts = np.bincount(bid, minlength=NCORES * NW)
    bmat = np.ceil(counts.reshape(NCORES, NW).max(axis=0) / 128.0).astype(np.int64)
    blockbase = np.zeros(NW, np.int64)
    np.cumsum(bmat[:-1], out=blockbase[1:])
    NBLK = int(bmat.sum())
    starts = np.zeros(NCORES * NW, np.int64)
    np.cumsum(counts[:-1], out=starts[1:])
    order = np.argsort(bid, kind="stable")
    rank = np.empty(ne, np.int64)
    rank[order] = np.arange(ne) - starts[bid[order]]
    j = blockbase[w] + rank // 128
    p = rank % 128
    colw = np.full((NCORES, 128, NBLK), -1.0, np.float32)
    colw[shard, p, j] = cw
    stream = np.zeros((NCORES, 128, NBLK, F), BF)
    stream[shard, p, j] = src_bf[row]
    out = {"stream": stream.reshape(NCORES, 128, NBLK * F),
           "colw": colw.astype(BF)}
    for name, vals in aux.items():
        a = np.zeros((NCORES, 128, NBLK), np.float32)
        a[shard, p, j] = vals.astype(np.float32)
        out[name] = a.astype(BF)
    meta = {"NBLK": NBLK, "bmat": bmat.tolist(), "blockbase": blockbase.tolist()}
    return out, meta


def host_prep(inputs):
    uf = np.asarray(inputs["u_feature"], dtype=np.float32)
    emb = np.asarray(inputs["emb_table"], dtype=np.float32)
    no_N = np.asarray(inputs["no_Nidx"]).astype(np.int64)
    e_tabs = {c: np.asarray(inputs[f"e{c}"], dtype=np.float32) for c in (0, 3, 7, 8, 9)}
    newF = np.concatenate(
        [
            e_tabs[0][uf[:, 0].astype(np.int64)],
            uf[:, 1:3],
            e_tabs[3][uf[:, 3].astype(np.int64)],
            uf[:, 4:7],
            e_tabs[7][uf[:, 7].astype(np.int64)],
            e_tabs[8][uf[:, 8].astype(np.int64)],
            e_tabs[9][uf[:, 9].astype(np.int64)],
        ],
        axis=1,
    )
    userF = np.concatenate([newF, emb[no_N]], axis=0)   # [100000, 85]

    ufp = np.zeros((NCORES * L + (LP - L), ED), np.float32)
    ufp[:U] = userF
    # pre-transposed local userF slice: [85, LP] for contiguous ufT loads
    ulocs = [np.ascontiguousarray(ufp[k * L : k * L + LP].T).astype(BF)
             for k in range(NCORES)]

    edge_uu = np.asarray(inputs["edge_uu"]).astype(np.int64)
    cu_src = np.asarray(inputs["edge_cu_src"]).astype(np.int64)
    cu_dst = np.asarray(inputs["edge_cu_dst"]).astype(np.int64)
    deg = np.bincount(edge_uu[1], minlength=U)
    cnt = np.bincount(cu_dst, minlength=U)

    userF_bf = userF.astype(BF)
    comment_bf = np.asarray(inputs["comment_x"], dtype=np.float32).astype(BF)

    uu_arr, uu_meta = _bucket_stream(
        edge_uu[0], edge_uu[1], userF_bf, ED,
        {"wdeg": deg[edge_uu[0]], "cdeg": deg[edge_uu[1]]},
    )
    cu_arr, cu_meta = _bucket_stream(
        cu_src, cu_dst, comment_bf, DC, {"wcnt": cnt[cu_dst]})

    iota8 = np.tile(np.arange(WIN, dtype=np.float32), (128, CHUNK)).astype(BF)
    ident = np.eye(128, dtype=np.float32)

    shared = {
        "iota8": iota8,
        "ident": ident,
        "wg": np.asarray(inputs["gcn_w"], np.float32).astype(BF),
        "wr": np.asarray(inputs["sage_r_w"], np.float32).astype(BF),
        "wl": np.asarray(inputs["sage_l_w"], np.float32).astype(BF),
        "gcn_b": np.asarray(inputs["gcn_b"], np.float32),
        "sage_l_b": np.asarray(inputs["sage_l_b"], np.float32),
        "bn_gamma": np.asarray(inputs["bn_gamma"], np.float32),
        "bn_beta": np.asarray(inputs["bn_beta"], np.float32),
    }
    percore = []
    for k in range(NCORES):
        m = dict(shared)
        m["uloc"] = ulocs[k]
        m["stream_u"] = uu_arr["stream"][k]
        m["colw_u"] = uu_arr["colw"][k]
        m["wdeg"] = uu_arr["wdeg"][k]
        m["cdeg"] = uu_arr["cdeg"][k]
        m["stream_c"] = cu_arr["stream"][k]
        m["colw_c"] = cu_arr["colw"][k]
        m["wcnt"] = cu_arr["wcnt"][k]
        percore.append(m)
    cfg = {"uu": uu_meta, "cu": cu_meta}
    return percore, cfg


def _win_flags(meta):
    """Per-block (window, first-of-window, last-of-window, last-of-GROUP)."""
    flags = []
    grp_last = {}
    for w in range(NW):
        b0, nb = meta["blockbase"][w], meta["bmat"][w]
        if nb:
            grp_last[w // GRP] = b0 + nb - 1
    for w in range(NW):
        b0, nb = meta["blockbase"][w], meta["bmat"][w]
        for b in range(nb):
            jg = b0 + b
            flags.append((w, b == 0, b == nb - 1, jg == grp_last[w // GRP]))
    empty_grps = [i for i in range(NW // GRP) if i not in grp_last]
    return flags, empty_grps


def build(nc, tc, io, out_ap, cfg):
    AT = mybir.AluOpType
    AF = mybir.ActivationFunctionType
    AX = mybir.AxisListType
    RG = [list(range(NCORES))]
    mu, mc = cfg["uu"], cfg["cu"]
    NBU, NBC = mu["NBLK"], mc["NBLK"]
    flags_u, empty_u = _win_flags(mu)
    flags_c, empty_c = _win_flags(mc)

    bn_inB = nc.dram_tensor("bn_inB", [H, 2], F32).ap()
    bn_outB = nc.dram_tensor("bn_outB", [H, 2], F32, addr_space="Shared").ap()

    import contextlib

    stack = contextlib.ExitStack()
    big = stack.enter_context(tc.tile_pool(name="big", bufs=1))
    iota8_sb = big.tile([128, CHUNK * WIN], BF16, tag="iota8")
    ident_sb = big.tile([128, 128], F32, tag="ident")
    wg_sb = [big.tile([ED, H], BF16, name=f"wg{h}", tag=f"wg{h}") for h in range(NH)]
    wr_sb = [big.tile([ED, H], BF16, name=f"wr{h}", tag=f"wr{h}") for h in range(NH)]
    wl_sb = [big.tile([DC, H], BF16, name=f"wl{h}", tag=f"wl{h}") for h in range(NH)]
    bh_sb = [big.tile([H, 1], F32, name=f"bh{h}", tag=f"bh{h}") for h in range(NH)]
    nbh_sb = [big.tile([H, 1], F32, name=f"nbh{h}", tag=f"nbh{h}") for h in range(NH)]
    gam_sb = big.tile([H, 1], F32, tag="gam")
    bet_sb = big.tile([H, 1], F32, tag="bet")
    colw_u_sb = big.tile([128, NBU], BF16, tag="colw_u")
    ec_u_sb = big.tile([128, NBU], BF16, tag="ec_u")
    colw_c_sb = big.tile([128, NBC], BF16, tag="colw_c")
    ci_c_sb = big.tile([128, NBC], BF16, tag="ci_c")
    agg_u = big.tile([ED, LP], BF16, tag="agg_u")
    agg_c = big.tile([DC, LP], BF16, tag="agg_c")
    node = big.tile([H, LP], BF16, tag="node")
    s_part = big.tile([H, NT], F32, tag="s_part")
    sq_part = big.tile([H, NT], F32, tag="sq_part")

    # coefficient inputs first: they gate the first S-builds
    nc.sync.dma_start(out=colw_u_sb[:], in_=io["colw_u"])
    nc.sync.dma_start(out=colw_c_sb[:], in_=io["colw_c"])

    # ---- per-edge coefficients ----------------------------------------
    # ec_u = dis(wdeg)*dis(cdeg), dis(x) = (x>0) * rsqrt(max(x,1))
    # ci_c = 1/max(wcnt, 1)
    coefp_cm = tc.tile_pool(name="coef", bufs=1)
    coefp = coefp_cm.__enter__()
    wdeg = coefp.tile([128, NBU], BF16, tag="wdeg")
    cdeg = coefp.tile([128, NBU], BF16, tag="cdeg")
    wcnt = coefp.tile([128, NBC], BF16, tag="wcnt")
    nc.sync.dma_start(out=wdeg[:], in_=io["wdeg"])
    nc.sync.dma_start(out=cdeg[:], in_=io["cdeg"])
    nc.sync.dma_start(out=wcnt[:], in_=io["wcnt"])
    d1 = coefp.tile([128, NBU], F32, tag="d1")
    d2 = coefp.tile([128, NBU], F32, tag="d2")
    for src, dst in ((wdeg, d1), (cdeg, d2)):
        mx = coefp.tile([128, NBU], F32, tag="mx")
        nc.vector.tensor_scalar(out=mx[:], in0=src[:], scalar1=1.0,
                                scalar2=None, op0=AT.max)
        rc = coefp.tile([128, NBU], F32, tag="rc")
        nc.vector.reciprocal_approx_fast(out=rc[:], in_=mx[:])
        rs = coefp.tile([128, NBU], F32, tag="rs")
        nc.scalar.activation(out=rs[:], in_=rc[:], func=AF.Sqrt)
        mk = coefp.tile([128, NBU], F32, tag="mk")
        nc.vector.tensor_scalar(out=mk[:], in0=src[:], scalar1=0.0,
                                scalar2=None, op0=AT.is_gt)
        nc.vector.tensor_tensor(out=dst[:], in0=rs[:], in1=mk[:], op=AT.mult)
    nc.vector.tensor_tensor(out=ec_u_sb[:], in0=d1[:], in1=d2[:], op=AT.mult)
    cmx = coefp.tile([128, NBC], F32, tag="cmx")
    nc.vector.tensor_scalar(out=cmx[:], in0=wcnt[:], scalar1=1.0,
                            scalar2=None, op0=AT.max)
    crc = coefp.tile([128, NBC], F32, tag="crc")
    nc.vector.reciprocal_approx_fast(out=crc[:], in_=cmx[:])
    nc.scalar.copy(out=ci_c_sb[:], in_=crc[:])
    coefp_cm.__exit__(None, None, None)

    nc.sync.dma_start(out=iota8_sb[:], in_=io["iota8"])
    nc.sync.dma_start(out=ident_sb[:], in_=io["ident"])
    for h in range(NH):
        nc.sync.dma_start(out=wg_sb[h][:], in_=io["wg"][h])
        nc.sync.dma_start(out=wr_sb[h][:], in_=io["wr"][h])
        nc.sync.dma_start(out=wl_sb[h][:], in_=io["wl"][h])
    nc.sync.dma_start(out=gam_sb[:], in_=io["bn_gamma"][:, None])
    nc.sync.dma_start(out=bet_sb[:], in_=io["bn_beta"][:, None])

    # ---- biases: bh = gcn_b + sage_l_b; nbh = -bh ----------------------
    with tc.tile_pool(name="bias", bufs=2) as bp:
        for h in range(NH):
            t1 = bp.tile([H, 1], F32, tag="t1")
            t2 = bp.tile([H, 1], F32, tag="t2")
            nc.sync.dma_start(out=t1[:], in_=io["gcn_b"][h][:, None])
            nc.sync.dma_start(out=t2[:], in_=io["sage_l_b"][h][:, None])
            nc.vector.tensor_tensor(out=bh_sb[h][:], in0=t1[:], in1=t2[:], op=AT.add)
            nc.vector.tensor_scalar(out=nbh_sb[h][:], in0=bh_sb[h][:],
                                    scalar1=-1.0, scalar2=None, op0=AT.mult)

    # ---- streamed one-hot matmul aggregation ---------------------------
    def chunk_list(nblk):
        return [(c0, min(CHUNK, nblk - c0)) for c0 in range(0, nblk, CHUNK)]

    chunks_u = chunk_list(NBU)
    chunks_c = chunk_list(NBC)

    # final tile t needs both aggs for windows <= min(16t+15, NW-1)
    def need_chunk(meta, w):
        last_blk = meta["blockbase"][w] + max(meta["bmat"][w], 1) - 1
        return last_blk // CHUNK

    fin_need = []
    for t in range(NT):
        wlast = min(16 * t + 15, NW - 1)
        fin_need.append((need_chunk(mu, wlast), need_chunk(mc, wlast)))

    fin_pool = stack.enter_context(tc.tile_pool(name="fin", bufs=2))
    finp_pool = stack.enter_context(tc.tile_pool(name="finp", bufs=2, space="PSUM"))
    bnst = stack.enter_context(tc.tile_pool(name="bnst", bufs=1))
    statB = bnst.tile([H, 2], F32, tag="statB")

    uft_map = {}

    def prefetch_uft(t):
        if t < NT and t not in uft_map:
            tp0 = t * FTILE
            tpn = min(FTILE, LP - tp0)
            ufT = fin_pool.tile([ED, FTILE], BF16, tag="ufT")
            nc.sync.dma_start(out=ufT[:, :tpn], in_=io["uloc"][:, tp0 : tp0 + tpn])
            uft_map[t] = ufT

    def emit_final_tile(t):
        t0 = t * FTILE
        tn = min(FTILE, LP - t0)
        prefetch_uft(t)
        ufT = uft_map.pop(t)
        prefetch_uft(t + 1)
        rel = []
        for h in range(NH):
            ph = finp_pool.tile([H, FTILE], F32, tag="ph")
            nc.tensor.matmul(out=ph[:, :tn], lhsT=wg_sb[h][:],
                             rhs=agg_u[:, t0 : t0 + tn], start=True, stop=False)
            nc.tensor.matmul(out=ph[:, :tn], lhsT=wr_sb[h][:],
                             rhs=ufT[:, :tn], start=False, stop=False)
            nc.tensor.matmul(out=ph[:, :tn], lhsT=wl_sb[h][:],
                             rhs=agg_c[:, t0 : t0 + tn], start=False, stop=True)
            rp = fin_pool.tile([H, FTILE], F32, tag="rp")
            nc.scalar.activation(out=rp[:, :tn], in_=ph[:, :tn], func=AF.Relu,
                                 bias=bh_sb[h][:])
            rn = fin_pool.tile([H, FTILE], F32, tag="rn")
            nc.scalar.activation(out=rn[:, :tn], in_=ph[:, :tn], func=AF.Relu,
                                 bias=nbh_sb[h][:], scale=-1.0)
            rel.append((rp, rn))
        add_eng = nc.vector
        a1 = fin_pool.tile([H, FTILE], F32, tag="a1")
        add_eng.tensor_tensor(out=a1[:, :tn], in0=rel[0][0][:, :tn],
                              in1=rel[1][0][:, :tn], op=AT.add)
        a2 = fin_pool.tile([H, FTILE], F32, tag="a2")
        add_eng.tensor_tensor(out=a2[:, :tn], in0=rel[0][1][:, :tn],
                              in1=rel[1][1][:, :tn], op=AT.add)
        # node = a1 - 0.3*a2  (leaky relu combine)
        nc.vector.scalar_tensor_tensor(
            out=node[:, t0 : t0 + tn], in0=a2[:, :tn], scalar=-0.3,
            in1=a1[:, :tn], op0=AT.mult, op1=AT.add)
        # BN stats; the last tile covers only real columns (pad rows are
        # discarded by the host, so they never need zeroing)
        sn = tn if t < NT - 1 else L - t0
        nc.vector.tensor_reduce(out=s_part[:, t : t + 1],
                                in_=node[:, t0 : t0 + sn], axis=AX.X, op=AT.add)
        sqs = fin_pool.tile([H, FTILE], F32, tag="sqs")
        nc.scalar.activation(out=sqs[:, :sn], in_=node[:, t0 : t0 + sn],
                             func=AF.Square, accum_out=sq_part[:, t : t + 1])
        if t == NT - 1:
            nc.vector.tensor_reduce(out=statB[:, 0:1], in_=s_part[:],
                                    axis=AX.X, op=AT.add)
            nc.vector.tensor_reduce(out=statB[:, 1:2], in_=sq_part[:],
                                    axis=AX.X, op=AT.add)
            nc.sync.dma_start(out=bn_inB, in_=statB[:])
            nc.gpsimd.collective_compute(
                "AllReduce", mybir.AluOpType.add, replica_groups=RG,
                ins=[bn_inB], outs=[bn_outB])

    with (
        tc.tile_pool(name="gu", bufs=4) as gup,
        tc.tile_pool(name="gc", bufs=4) as gcp,
        tc.tile_pool(name="sp", bufs=4) as sp,
        tc.tile_pool(name="aggp", bufs=6, space="PSUM") as aggp,
    ):
        for i in empty_u:
            nc.vector.memset(agg_u[:, i * 128 : (i + 1) * 128], 0.0)
        for i in empty_c:
            nc.vector.memset(agg_c[:, i * 128 : (i + 1) * 128], 0.0)

        pm_open = {}

        def emit_chunk(relname, c0, nb, io_s, F, colw_sb, coef_sb, agg, rows,
                       flags, gpool, meta, dma_eng, s2_eng):
            g = gpool.tile([128, CHUNK * F], BF16, tag=f"g_{relname}")
            dma_eng.dma_start(out=g[:, : nb * F],
                              in_=io_s[:, c0 * F : (c0 + nb) * F])
            T = sp.tile([128, CHUNK * WIN], BF16, tag=f"T_{relname}")
            S = sp.tile([128, CHUNK * WIN], BF16, tag=f"S_{relname}")
            cb = colw_sb[:, c0 : c0 + nb].unsqueeze(-1).broadcast_to([128, nb, WIN])
            eb = coef_sb[:, c0 : c0 + nb].unsqueeze(-1).broadcast_to([128, nb, WIN])
            nc.vector.tensor_tensor(
                out=T[:, : nb * WIN].rearrange("p (c e) -> p c e", e=WIN),
                in0=iota8_sb[:, : nb * WIN].rearrange("p (c e) -> p c e", e=WIN),
                in1=cb, op=AT.is_equal)
            s2_eng.tensor_tensor(
                out=S[:, : nb * WIN].rearrange("p (c e) -> p c e", e=WIN),
                in0=T[:, : nb * WIN].rearrange("p (c e) -> p c e", e=WIN),
                in1=eb, op=AT.mult)
            for jj in range(nb):
                jg = c0 + jj
                w, first, wlast, glast = flags[jg]
                grp = w // GRP
                half = w % GRP
                key = (relname, grp)
                if key not in pm_open:
                    pm_open[key] = aggp.tile([128, GRP * WIN], F32, tag="pm",
                                             name=f"pm_{relname}_{grp}")
                pm = pm_open[key]
                nc.tensor.matmul(
                    out=pm[:rows, half * WIN : (half + 1) * WIN],
                    lhsT=g[:, jj * F : (jj + 1) * F],
                    rhs=S[:, jj * WIN : (jj + 1) * WIN],
                    start=first, stop=wlast)
                if glast:
                    if all(meta["bmat"][GRP * grp + hw] for hw in range(GRP)):
                        nc.scalar.copy(out=agg[:, grp * 128 : (grp + 1) * 128],
                                       in_=pm[:rows, :])
                    else:
                        for hw in range(GRP):
                            sl = agg[:, grp * 128 + hw * WIN :
                                     grp * 128 + (hw + 1) * WIN]
                            if meta["bmat"][GRP * grp + hw] == 0:
                                nc.vector.memset(sl, 0.0)
                            else:
                                nc.scalar.copy(
                                    out=sl,
                                    in_=pm[:rows, hw * WIN : (hw + 1) * WIN])
                    del pm_open[key]

        emitted_fin = 0
        nchunks = max(len(chunks_u), len(chunks_c))
        for ci in range(nchunks):
            if ci < len(chunks_u):
                c0, nb = chunks_u[ci]
                emit_chunk("u", c0, nb, io["stream_u"], ED, colw_u_sb, ec_u_sb,
                           agg_u, ED, flags_u, gup, mu, nc.sync, nc.gpsimd)
            if ci < len(chunks_c):
                c0, nb = chunks_c[ci]
                emit_chunk("c", c0, nb, io["stream_c"], DC, colw_c_sb, ci_c_sb,
                           agg_c, DC, flags_c, gcp, mc, nc.scalar, nc.vector)
            LAG = 2   # margin so final-tile matmuls never head-block the
                      # in-order PE queue waiting on fresh agg copies
            while (emitted_fin < NT
                   and ((fin_need[emitted_fin][0] + LAG <= ci
                         and fin_need[emitted_fin][1] + LAG <= ci)
                        or ci == nchunks - 1)):
                emit_final_tile(emitted_fin)
                emitted_fin += 1
        assert emitted_fin == NT, (emitted_fin, NT)

    # ---- BN: allreduce stats, normalize, transpose out -----------------
    with (
        tc.tile_pool(name="bn", bufs=3) as bn,
        tc.tile_pool(name="bnp", bufs=2, space="PSUM") as bnp,
    ):
        gstat = bn.tile([H, 2], F32, tag="gstat")
        nc.sync.dma_start(out=gstat[:], in_=bn_outB)
        mean = bn.tile([H, 1], F32, tag="mean")
        nc.vector.tensor_scalar(out=mean[:], in0=gstat[:, 0:1], scalar1=1.0 / U,
                                scalar2=None, op0=AT.mult)
        ex2 = bn.tile([H, 1], F32, tag="ex2")
        nc.vector.tensor_scalar(out=ex2[:], in0=gstat[:, 1:2], scalar1=1.0 / U,
                                scalar2=None, op0=AT.mult)
        m2 = bn.tile([H, 1], F32, tag="m2")
        nc.vector.tensor_tensor(out=m2[:], in0=mean[:], in1=mean[:], op=AT.mult)
        var = bn.tile([H, 1], F32, tag="var")
        nc.vector.tensor_tensor(out=var[:], in0=ex2[:], in1=m2[:], op=AT.subtract)
        vd = bn.tile([H, 1], F32, tag="vd")
        nc.vector.tensor_scalar(out=vd[:], in0=var[:], scalar1=1e-5, scalar2=None,
                                op0=AT.add)
        rv = bn.tile([H, 1], F32, tag="rv")
        nc.vector.reciprocal_approx_fast(out=rv[:], in_=vd[:])
        rs = bn.tile([H, 1], F32, tag="rs")
        nc.scalar.activation(out=rs[:], in_=rv[:], func=AF.Sqrt)
        asc = bn.tile([H, 1], F32, tag="asc")
        nc.vector.tensor_tensor(out=asc[:], in0=rs[:], in1=gam_sb[:], op=AT.mult)
        mb = bn.tile([H, 1], F32, tag="mb")
        nc.vector.tensor_tensor(out=mb[:], in0=mean[:], in1=asc[:], op=AT.mult)
        bsh = bn.tile([H, 1], F32, tag="bsh")
        nc.vector.tensor_tensor(out=bsh[:], in0=bet_sb[:], in1=mb[:], op=AT.subtract)
        OG = 8
        for n0 in range(0, NOW, OG):
            gn = min(OG, NOW - n0)
            yt8 = bn.tile([H, OG * 128], F32, tag="yt8")
            nc.vector.tensor_scalar(
                out=yt8[:, : gn * 128], in0=node[:, n0 * 128 : (n0 + gn) * 128],
                scalar1=asc[:], scalar2=bsh[:], op0=AT.mult, op1=AT.add)
            pt8 = bnp.tile([128, OG * H], F32, tag="pt8")
            for gi in range(gn):
                nc.tensor.transpose(out=pt8[:, gi * H : (gi + 1) * H],
                                    in_=yt8[:, gi * 128 : (gi + 1) * 128],
                                    identity=ident_sb[:])
            stg = bn.tile([128, OG * H], BF16, tag="stg")
            nc.scalar.activation(out=stg[:, : gn * H], in_=pt8[:, : gn * H],
                                 func=AF.Copy)
            eng = nc.scalar if (n0 // OG) % 2 else nc.sync
            eng.dma_start(out=out_ap[:, n0 * H : (n0 + gn) * H],
                          in_=stg[:, : gn * H])

    stack.close()


def make_nc(cfg):
    mu, mc = cfg["uu"], cfg["cu"]
    nc = bacc.Bacc(
        "TRN2",
        target_bir_lowering=False,
        debug=False,
        enable_asserts=False,
        num_devices=NCORES,
    )
    io = {}
    specs = [
        ("stream_u", (128, mu["NBLK"] * ED), BF16),
        ("stream_c", (128, mc["NBLK"] * DC), BF16),
        ("uloc", (ED, LP), BF16),
        ("iota8", (128, CHUNK * WIN), BF16),
        ("ident", (128, 128), F32),
        ("wg", (NH, ED, H), BF16),
        ("wr", (NH, ED, H), BF16),
        ("wl", (NH, DC, H), BF16),
        ("gcn_b", (NH, H), F32),
        ("sage_l_b", (NH, H), F32),
        ("bn_gamma", (H,), F32),
        ("bn_beta", (H,), F32),
        ("colw_u", (128, mu["NBLK"]), BF16),
        ("wdeg", (128, mu["NBLK"]), BF16),
        ("cdeg", (128, mu["NBLK"]), BF16),
        ("colw_c", (128, mc["NBLK"]), BF16),
        ("wcnt", (128, mc["NBLK"]), BF16),
    ]
    for name, shape, dt in specs:
        io[name] = nc.dram_tensor(name, list(shape), dt, kind="ExternalInput").ap()
    # output: [128, NOW*H] bf16, partition-contiguous; host relayouts
    out_ap = nc.dram_tensor("out_shard", [128, NOW * H], BF16,
                            kind="ExternalOutput").ap()
    with tile.TileContext(nc) as tc:
        build(nc, tc, io, out_ap, cfg)
    nc.compile()
    return nc


def kernel(**inputs):
    percore, cfg = host_prep(inputs)
    nc = make_nc(cfg)
    res = bass_utils.run_bass_kernel_spmd(nc, percore, core_ids=list(range(NCORES)))
    outs = []
    for k in range(NCORES):
        o = np.asarray(res.results[k]["out_shard"])      # [128, NOW*H] bf16
        o = o.reshape(128, NOW, H).transpose(1, 0, 2).reshape(LP, H)
        outs.append(o[:L])
    return np.concatenate(outs, axis=0).astype(np.float32)


# revision 31
# speedup vs baseline: 1.0177x; 1.0177x over previous
"""Trainium2 Bass kernel for the 2-hop GNN (GCN + SAGE + BatchNorm).

Strategy (8 NeuronCores, SPMD, destination sharding):
  - Core k owns output rows [k*12500, (k+1)*12500); padded to 12544 = 392
    aggregation windows of 32 destinations (output staged per 128).
  - Host prep is pure indexing / integer work: assemble userF by embedding
    lookups+concat (gathers, no arithmetic), bincount degrees/counts, sort
    edges by destination window, and lay the per-edge source rows out as a
    dense window-major stream per core (the per-input edge list is known at
    compile time, so the random-access gather is baked into the stream
    layout; no SWDGE descriptor generation on device).  All FP arithmetic
    runs on device.
  - Device: sequential-stream the edge rows (big contiguous HWDGE DMAs on
    two queues), build S[e,d] = (col[e]==d) * coef[e] in 24-block batches:
    one-hot is_equal on DVE, coef broadcast-mult on GpSimd.  Segment-sum
    via one-hot matmuls accumulating per 32-dest window into 4-window PSUM
    tiles; one PSUM->SBUF copy per 128-dest group on the Scalar engine.
    coef folds dis[row]*dis[col] (resp. 1/cnt[dst]) computed on device
    from uploaded integer degree values.
  - Final: per 512-dest tile, 3 bf16 matmuls per hop + leaky relu
    (relu-pair trick: Relu on ACT, hop-sums on GpSimd, fused combine on
    DVE); final tiles interleave with the streams.  BN stats AllReduced
    once right after the last tile (last GpSimd queue entry, so nothing
    stalls behind the collective); output staged per 8 windows and written
    contiguous bf16 (host relayouts to [N, H] f32).
"""

import numpy as np
import ml_dtypes

import concourse.bass as bass
import concourse.bacc as bacc
import concourse.tile as tile
import concourse.mybir as mybir
from concourse import bass_utils

F32 = mybir.dt.float32
BF16 = mybir.dt.bfloat16

U1 = 50000
U2 = 50000
U = 100000
C = 200000
E = 1000000
ED = 85
DC = 64
H = 128
NH = 2

NCORES = 8
L = 12500
WIN = 32               # aggregation window (dests per one-hot matmul)
NW = 392               # LP / WIN
GRP = 128 // WIN       # windows per PSUM tile / agg copy
LP = NW * WIN          # 12544
OW = 128               # output-stage window
NOW = 98               # LP / OW
CHUNK = 32             # stream blocks per DMA / S-build batch
FTILE = 512
NT = (LP + FTILE - 1) // FTILE   # 25
SPLIT_T = NT           # single end-of-stream stats AllReduce
BF = ml_dtypes.bfloat16


def _bucket_stream(row, col, src_bf, F, aux):
    """Sort edges by (dest core, dest window); emit per-core dense streams.

    Returns per-core arrays {stream, colw, aux...} plus shared block meta.
    The block structure (bmat/blockbase) is shared across cores (SPMD
    program), padded to the worst core per window.
    """
    ne = len(row)
    shard = col // L
    lc = col % L
    w = lc // WIN
    cw = (lc % WIN).astype(np.float32)
    bid = shard * NW + w
    counts = np.bincount(bid, minlength=NCORES * NW)
    bmat = np.ceil(counts.reshape(NCORES, NW).max(axis=0) / 128.0).astype(np.int64)
    blockbase = np.zeros(NW, np.int64)
    np.cumsum(bmat[:-1], out=blockbase[1:])
    NBLK = int(bmat.sum())
    starts = np.zeros(NCORES * NW, np.int64)
    np.cumsum(counts[:-1], out=starts[1:])
    order = np.argsort(bid, kind="stable")
    rank = np.empty(ne, np.int64)
    rank[order] = np.arange(ne) - starts[bid[order]]
    j = blockbase[w] + rank // 128
    p = rank % 128
    colw = np.full((NCORES, 128, NBLK), -1.0, np.float32)
    colw[shard, p, j] = cw
    stream = np.zeros((NCORES, 128, NBLK, F), BF)
    stream[shard, p, j] = src_bf[row]
    out = {"stream": stream.reshape(NCORES, 128, NBLK * F),
           "colw": colw.astype(BF)}
    for name, vals in aux.items():
        a = np.zeros((NCORES, 128, NBLK), np.float32)
        a[shard, p, j] = vals.astype(np.float32)
        out[name] = a.astype(BF)
    meta = {"NBLK": NBLK, "bmat": bmat.tolist(), "blockbase": blockbase.tolist()}
    return out, meta


def host_prep(inputs):
    uf = np.asarray(inputs["u_feature"], dtype=np.float32)
    emb = np.asarray(inputs["emb_table"], dtype=np.float32)
    no_N = np.asarray(inputs["no_Nidx"]).astype(np.int64)
    e_tabs = {c: np.asarray(inputs[f"e{c}"], dtype=np.float32) for c in (0, 3, 7, 8, 9)}
    newF = np.concatenate(
        [
            e_tabs[0][uf[:, 0].astype(np.int64)],
            uf[:, 1:3],
            e_tabs[3][uf[:, 3].astype(np.int64)],
            uf[:, 4:7],
            e_tabs[7][uf[:, 7].astype(np.int64)],
            e_tabs[8][uf[:, 8].astype(np.int64)],
            e_tabs[9][uf[:, 9].astype(np.int64)],
        ],
        axis=1,
    )
    userF = np.concatenate([newF, emb[no_N]], axis=0)   # [100000, 85]

    ufp = np.zeros((NCORES * L + (LP - L), ED), np.float32)
    ufp[:U] = userF
    # pre-transposed local userF slice: [85, LP] for contiguous ufT loads
    ulocs = [np.ascontiguousarray(ufp[k * L : k * L + LP].T).astype(BF)
             for k in range(NCORES)]

    edge_uu = np.asarray(inputs["edge_uu"]).astype(np.int64)
    cu_src = np.asarray(inputs["edge_cu_src"]).astype(np.int64)
    cu_dst = np.asarray(inputs["edge_cu_dst"]).astype(np.int64)
    deg = np.bincount(edge_uu[1], minlength=U)
    cnt = np.bincount(cu_dst, minlength=U)

    userF_bf = userF.astype(BF)
    comment_bf = np.asarray(inputs["comment_x"], dtype=np.float32).astype(BF)

    uu_arr, uu_meta = _bucket_stream(
        edge_uu[0], edge_uu[1], userF_bf, ED,
        {"wdeg": deg[edge_uu[0]], "cdeg": deg[edge_uu[1]]},
    )
    cu_arr, cu_meta = _bucket_stream(
        cu_src, cu_dst, comment_bf, DC, {"wcnt": cnt[cu_dst]})

    iota8 = np.tile(np.arange(WIN, dtype=np.float32), (128, CHUNK)).astype(BF)
    ident = np.eye(128, dtype=np.float32)

    shared = {
        "iota8": iota8,
        "ident": ident,
        "wg": np.asarray(inputs["gcn_w"], np.float32).astype(BF),
        "wr": np.asarray(inputs["sage_r_w"], np.float32).astype(BF),
        "wl": np.asarray(inputs["sage_l_w"], np.float32).astype(BF),
        "gcn_b": np.asarray(inputs["gcn_b"], np.float32),
        "sage_l_b": np.asarray(inputs["sage_l_b"], np.float32),
        "bn_gamma": np.asarray(inputs["bn_gamma"], np.float32),
        "bn_beta": np.asarray(inputs["bn_beta"], np.float32),
    }
    percore = []
    for k in range(NCORES):
        m = dict(shared)
        m["uloc"] = ulocs[k]
        m["stream_u"] = uu_arr["stream"][k]
        m["colw_u"] = uu_arr["colw"][k]
        m["wdeg"] = uu_arr["wdeg"][k]
        m["cdeg"] = uu_arr["cdeg"][k]
        m["stream_c"] = cu_arr["stream"][k]
        m["colw_c"] = cu_arr["colw"][k]
        m["wcnt"] = cu_arr["wcnt"][k]
        percore.append(m)
    cfg = {"uu": uu_meta, "cu": cu_meta}
    return percore, cfg


def _win_flags(meta):
    """Per-block (window, first-of-window, last-of-window, last-of-GROUP)."""
    flags = []
    grp_last = {}
    for w in range(NW):
        b0, nb = meta["blockbase"][w], meta["bmat"][w]
        if nb:
            grp_last[w // GRP] = b0 + nb - 1
    for w in range(NW):
        b0, nb = meta["blockbase"][w], meta["bmat"][w]
        for b in range(nb):
            jg = b0 + b
            flags.append((w, b == 0, b == nb - 1, jg == grp_last[w // GRP]))
    empty_grps = [i for i in range(NW // GRP) if i not in grp_last]
    return flags, empty_grps


def build(nc, tc, io, out_ap, cfg):
    AT = mybir.AluOpType
    AF = mybir.ActivationFunctionType
    AX = mybir.AxisListType
    RG = [list(range(NCORES))]
    mu, mc = cfg["uu"], cfg["cu"]
    NBU, NBC = mu["NBLK"], mc["NBLK"]
    flags_u, empty_u = _win_flags(mu)
    flags_c, empty_c = _win_flags(mc)

    bn_inB = nc.dram_tensor("bn_inB", [H, 2], F32).ap()
    bn_outB = nc.dram_tensor("bn_outB", [H, 2], F32, addr_space="Shared").ap()

    import contextlib

    stack = contextlib.ExitStack()
    big = stack.enter_context(tc.tile_pool(name="big", bufs=1))
    iota8_sb = big.tile([128, CHUNK * WIN], BF16, tag="iota8")
    ident_sb = big.tile([128, 128], F32, tag="ident")
    wg_sb = [big.tile([ED, H], BF16, name=f"wg{h}", tag=f"wg{h}") for h in range(NH)]
    wr_sb = [big.tile([ED, H], BF16, name=f"wr{h}", tag=f"wr{h}") for h in range(NH)]
    wl_sb = [big.tile([DC, H], BF16, name=f"wl{h}", tag=f"wl{h}") for h in range(NH)]
    bh_sb = [big.tile([H, 1], F32, name=f"bh{h}", tag=f"bh{h}") for h in range(NH)]
    nbh_sb = [big.tile([H, 1], F32, name=f"nbh{h}", tag=f"nbh{h}") for h in range(NH)]
    gam_sb = big.tile([H, 1], F32, tag="gam")
    bet_sb = big.tile([H, 1], F32, tag="bet")
    colw_u_sb = big.tile([128, NBU], BF16, tag="colw_u")
    ec_u_sb = big.tile([128, NBU], BF16, tag="ec_u")
    colw_c_sb = big.tile([128, NBC], BF16, tag="colw_c")
    ci_c_sb = big.tile([128, NBC], BF16, tag="ci_c")
    agg_u = big.tile([ED, LP], BF16, tag="agg_u")
    agg_c = big.tile([DC, LP], BF16, tag="agg_c")
    node = big.tile([H, LP], BF16, tag="node")
    s_part = big.tile([H, NT], F32, tag="s_part")
    sq_part = big.tile([H, NT], F32, tag="sq_part")

    # coefficient inputs first: they gate the first S-builds
    nc.sync.dma_start(out=colw_u_sb[:], in_=io["colw_u"])
    nc.sync.dma_start(out=colw_c_sb[:], in_=io["colw_c"])

    # ---- per-edge coefficients ----------------------------------------
    # ec_u = dis(wdeg)*dis(cdeg), dis(x) = (x>0) * rsqrt(max(x,1))
    # ci_c = 1/max(wcnt, 1)
    coefp_cm = tc.tile_pool(name="coef", bufs=1)
    coefp = coefp_cm.__enter__()
    wdeg = coefp.tile([128, NBU], BF16, tag="wdeg")
    cdeg = coefp.tile([128, NBU], BF16, tag="cdeg")
    wcnt = coefp.tile([128, NBC], BF16, tag="wcnt")
    nc.sync.dma_start(out=wdeg[:], in_=io["wdeg"])
    nc.sync.dma_start(out=cdeg[:], in_=io["cdeg"])
    nc.sync.dma_start(out=wcnt[:], in_=io["wcnt"])
    d1 = coefp.tile([128, NBU], F32, tag="d1")
    d2 = coefp.tile([128, NBU], F32, tag="d2")
    for src, dst in ((wdeg, d1), (cdeg, d2)):
        mx = coefp.tile([128, NBU], F32, tag="mx")
        nc.vector.tensor_scalar(out=mx[:], in0=src[:], scalar1=1.0,
                                scalar2=None, op0=AT.max)
        rc = coefp.tile([128, NBU], F32, tag="rc")
        nc.vector.reciprocal_approx_fast(out=rc[:], in_=mx[:])
        rs = coefp.tile([128, NBU], F32, tag="rs")
        nc.scalar.activation(out=rs[:], in_=rc[:], func=AF.Sqrt)
        mk = coefp.tile([128, NBU], F32, tag="mk")
        nc.vector.tensor_scalar(out=mk[:], in0=src[:], scalar1=0.0,
                                scalar2=None, op0=AT.is_gt)
        nc.vector.tensor_tensor(out=dst[:], in0=rs[:], in1=mk[:], op=AT.mult)
    nc.vector.tensor_tensor(out=ec_u_sb[:], in0=d1[:], in1=d2[:], op=AT.mult)
    cmx = coefp.tile([128, NBC], F32, tag="cmx")
    nc.vector.tensor_scalar(out=cmx[:], in0=wcnt[:], scalar1=1.0,
                            scalar2=None, op0=AT.max)
    crc = coefp.tile([128, NBC], F32, tag="crc")
    nc.vector.reciprocal_approx_fast(out=crc[:], in_=cmx[:])
    nc.scalar.copy(out=ci_c_sb[:], in_=crc[:])
    coefp_cm.__exit__(None, None, None)

    nc.sync.dma_start(out=iota8_sb[:], in_=io["iota8"])
    nc.sync.dma_start(out=ident_sb[:], in_=io["ident"])
    for h in range(NH):
        nc.sync.dma_start(out=wg_sb[h][:], in_=io["wg"][h])
        nc.sync.dma_start(out=wr_sb[h][:], in_=io["wr"][h])
        nc.sync.dma_start(out=wl_sb[h][:], in_=io["wl"][h])
    nc.sync.dma_start(out=gam_sb[:], in_=io["bn_gamma"][:, None])
    nc.sync.dma_start(out=bet_sb[:], in_=io["bn_beta"][:, None])

    # ---- biases: bh = gcn_b + sage_l_b; nbh = -bh ----------------------
    with tc.tile_pool(name="bias", bufs=2) as bp:
        for h in range(NH):
            t1 = bp.tile([H, 1], F32, tag="t1")
            t2 = bp.tile([H, 1], F32, tag="t2")
            nc.sync.dma_start(out=t1[:], in_=io["gcn_b"][h][:, None])
            nc.sync.dma_start(out=t2[:], in_=io["sage_l_b"][h][:, None])
            nc.vector.tensor_tensor(out=bh_sb[h][:], in0=t1[:], in1=t2[:], op=AT.add)
            nc.vector.tensor_scalar(out=nbh_sb[h][:], in0=bh_sb[h][:],
                                    scalar1=-1.0, scalar2=None, op0=AT.mult)

    # ---- streamed one-hot matmul aggregation ---------------------------
    def chunk_list(nblk):
        return [(c0, min(CHUNK, nblk - c0)) for c0 in range(0, nblk, CHUNK)]

    chunks_u = chunk_list(NBU)
    chunks_c = chunk_list(NBC)

    # final tile t needs both aggs for windows <= min(16t+15, NW-1)
    def need_chunk(meta, w):
        last_blk = meta["blockbase"][w] + max(meta["bmat"][w], 1) - 1
        return last_blk // CHUNK

    fin_need = []
    for t in range(NT):
        wlast = min(16 * t + 15, NW - 1)
        fin_need.append((need_chunk(mu, wlast), need_chunk(mc, wlast)))

    fin_pool = stack.enter_context(tc.tile_pool(name="fin", bufs=2))
    finp_pool = stack.enter_context(tc.tile_pool(name="finp", bufs=2, space="PSUM"))
    bnst = stack.enter_context(tc.tile_pool(name="bnst", bufs=1))
    statB = bnst.tile([H, 2], F32, tag="statB")

    uft_map = {}

    def prefetch_uft(t):
        if t < NT and t not in uft_map:
            tp0 = t * FTILE
            tpn = min(FTILE, LP - tp0)
            ufT = fin_pool.tile([ED, FTILE], BF16, tag="ufT")
            nc.sync.dma_start(out=ufT[:, :tpn], in_=io["uloc"][:, tp0 : tp0 + tpn])
            uft_map[t] = ufT

    def emit_final_tile(t):
        t0 = t * FTILE
        tn = min(FTILE, LP - t0)
        prefetch_uft(t)
        ufT = uft_map.pop(t)
        prefetch_uft(t + 1)
        rel = []
        for h in range(NH):
            ph = finp_pool.tile([H, FTILE], F32, tag="ph")
            nc.tensor.matmul(out=ph[:, :tn], lhsT=wg_sb[h][:],
                             rhs=agg_u[:, t0 : t0 + tn], start=True, stop=False)
            nc.tensor.matmul(out=ph[:, :tn], lhsT=wr_sb[h][:],
                             rhs=ufT[:, :tn], start=False, stop=False)
            nc.tensor.matmul(out=ph[:, :tn], lhsT=wl_sb[h][:],
                             rhs=agg_c[:, t0 : t0 + tn], start=False, stop=True)
            rp = fin_pool.tile([H, FTILE], F32, tag="rp")
            nc.scalar.activation(out=rp[:, :tn], in_=ph[:, :tn], func=AF.Relu,
                                 bias=bh_sb[h][:])
            rn = fin_pool.tile([H, FTILE], F32, tag="rn")
            nc.scalar.activation(out=rn[:, :tn], in_=ph[:, :tn], func=AF.Relu,
                                 bias=nbh_sb[h][:], scale=-1.0)
            rel.append((rp, rn))
        add_eng = nc.vector
        a1 = fin_pool.tile([H, FTILE], F32, tag="a1")
        add_eng.tensor_tensor(out=a1[:, :tn], in0=rel[0][0][:, :tn],
                              in1=rel[1][0][:, :tn], op=AT.add)
        a2 = fin_pool.tile([H, FTILE], F32, tag="a2")
        add_eng.tensor_tensor(out=a2[:, :tn], in0=rel[0][1][:, :tn],
                              in1=rel[1][1][:, :tn], op=AT.add)
        # node = a1 - 0.3*a2  (leaky relu combine)
        nc.vector.scalar_tensor_tensor(
            out=node[:, t0 : t0 + tn], in0=a2[:, :tn], scalar=-0.3,
            in1=a1[:, :tn], op0=AT.mult, op1=AT.add)
        # BN stats; the last tile covers only real columns (pad rows are
        # discarded by the host, so they never need zeroing)
        sn = tn if t < NT - 1 else L - t0
        nc.vector.tensor_reduce(out=s_part[:, t : t + 1],
                                in_=node[:, t0 : t0 + sn], axis=AX.X, op=AT.add)
        sqs = fin_pool.tile([H, FTILE], F32, tag="sqs")
        nc.scalar.activation(out=sqs[:, :sn], in_=node[:, t0 : t0 + sn],
                             func=AF.Square, accum_out=sq_part[:, t : t + 1])
        if t == NT - 1:
            nc.vector.tensor_reduce(out=statB[:, 0:1], in_=s_part[:],
                                    axis=AX.X, op=AT.add)
            nc.vector.tensor_reduce(out=statB[:, 1:2], in_=sq_part[:],
                                    axis=AX.X, op=AT.add)
            nc.sync.dma_start(out=bn_inB, in_=statB[:])
            nc.gpsimd.collective_compute(
                "AllReduce", mybir.AluOpType.add, replica_groups=RG,
                ins=[bn_inB], outs=[bn_outB])

    with (
        tc.tile_pool(name="gu", bufs=4) as gup,
        tc.tile_pool(name="gc", bufs=4) as gcp,
        tc.tile_pool(name="sp", bufs=4) as sp,
        tc.tile_pool(name="aggp", bufs=6, space="PSUM") as aggp,
    ):
        for i in empty_u:
            nc.vector.memset(agg_u[:, i * 128 : (i + 1) * 128], 0.0)
        for i in empty_c:
            nc.vector.memset(agg_c[:, i * 128 : (i + 1) * 128], 0.0)

        pm_open = {}

        def emit_chunk(relname, c0, nb, io_s, F, colw_sb, coef_sb, agg, rows,
                       flags, gpool, meta, dma_eng, s2_eng):
            g = gpool.tile([128, CHUNK * F], BF16, tag=f"g_{relname}")
            dma_eng.dma_start(out=g[:, : nb * F],
                              in_=io_s[:, c0 * F : (c0 + nb) * F])
            T = sp.tile([128, CHUNK * WIN], BF16, tag=f"T_{relname}")
            S = sp.tile([128, CHUNK * WIN], BF16, tag=f"S_{relname}")
            cb = colw_sb[:, c0 : c0 + nb].unsqueeze(-1).broadcast_to([128, nb, WIN])
            eb = coef_sb[:, c0 : c0 + nb].unsqueeze(-1).broadcast_to([128, nb, WIN])
            nc.vector.tensor_tensor(
                out=T[:, : nb * WIN].rearrange("p (c e) -> p c e", e=WIN),
                in0=iota8_sb[:, : nb * WIN].rearrange("p (c e) -> p c e", e=WIN),
                in1=cb, op=AT.is_equal)
            s2_eng.tensor_tensor(
                out=S[:, : nb * WIN].rearrange("p (c e) -> p c e", e=WIN),
                in0=T[:, : nb * WIN].rearrange("p (c e) -> p c e", e=WIN),
                in1=eb, op=AT.mult)
            for jj in range(nb):
                jg = c0 + jj
                w, first, wlast, glast = flags[jg]
                grp = w // GRP
                half = w % GRP
                key = (relname, grp)
                if key not in pm_open:
                    pm_open[key] = aggp.tile([128, GRP * WIN], F32, tag="pm",
                                             name=f"pm_{relname}_{grp}")
                pm = pm_open[key]
                nc.tensor.matmul(
                    out=pm[:rows, half * WIN : (half + 1) * WIN],
                    lhsT=g[:, jj * F : (jj + 1) * F],
                    rhs=S[:, jj * WIN : (jj + 1) * WIN],
                    start=first, stop=wlast)
                if glast:
                    if all(meta["bmat"][GRP * grp + hw] for hw in range(GRP)):
                        nc.scalar.copy(out=agg[:, grp * 128 : (grp + 1) * 128],
                                       in_=pm[:rows, :])
                    else:
                        for hw in range(GRP):
                            sl = agg[:, grp * 128 + hw * WIN :
                                     grp * 128 + (hw + 1) * WIN]
                            if meta["bmat"][GRP * grp + hw] == 0:
                                nc.vector.memset(sl, 0.0)
                            else:
                                nc.scalar.copy(
                                    out=sl,
                                    in_=pm[:rows, hw * WIN : (hw + 1) * WIN])
                    del pm_open[key]

        emitted_fin = 0
        nchunks = max(len(chunks_u), len(chunks_c))
        for ci in range(nchunks):
            if ci < len(chunks_u):
                c0, nb = chunks_u[ci]
                emit_chunk("u", c0, nb, io["stream_u"], ED, colw_u_sb, ec_u_sb,
                           agg_u, ED, flags_u, gup, mu, nc.sync, nc.gpsimd)
            if ci < len(chunks_c):
                c0, nb = chunks_c[ci]
                emit_chunk("c", c0, nb, io["stream_c"], DC, colw_c_sb, ci_c_sb,
                           agg_c, DC, flags_c, gcp, mc, nc.scalar, nc.vector)
            LAG = 2   # margin so final-tile matmuls never head-block the
                      # in-order PE queue waiting on fresh agg copies
            while (emitted_fin < NT
                   and ((fin_need[emitted_fin][0] + LAG <= ci
                         and fin_need[emitted_fin][1] + LAG <= ci)
                        or ci == nchunks - 1)):
                emit_final_tile(emitted_fin)
                emitted_fin += 1
        assert emitted_fin == NT, (emitted_fin, NT)

    # ---- BN: allreduce stats, normalize, transpose out -----------------
    with (
        tc.tile_pool(name="bn", bufs=3) as bn,
        tc.tile_pool(name="bnp", bufs=2, space="PSUM") as bnp,
    ):
        gstat = bn.tile([H, 2], F32, tag="gstat")
        nc.sync.dma_start(out=gstat[:], in_=bn_outB)
        mean = bn.tile([H, 1], F32, tag="mean")
        nc.vector.tensor_scalar(out=mean[:], in0=gstat[:, 0:1], scalar1=1.0 / U,
                                scalar2=None, op0=AT.mult)
        ex2 = bn.tile([H, 1], F32, tag="ex2")
        nc.vector.tensor_scalar(out=ex2[:], in0=gstat[:, 1:2], scalar1=1.0 / U,
                                scalar2=None, op0=AT.mult)
        m2 = bn.tile([H, 1], F32, tag="m2")
        nc.vector.tensor_tensor(out=m2[:], in0=mean[:], in1=mean[:], op=AT.mult)
        var = bn.tile([H, 1], F32, tag="var")
        nc.vector.tensor_tensor(out=var[:], in0=ex2[:], in1=m2[:], op=AT.subtract)
        vd = bn.tile([H, 1], F32, tag="vd")
        nc.vector.tensor_scalar(out=vd[:], in0=var[:], scalar1=1e-5, scalar2=None,
                                op0=AT.add)
        rv = bn.tile([H, 1], F32, tag="rv")
        nc.vector.reciprocal_approx_fast(out=rv[:], in_=vd[:])
        rs = bn.tile([H, 1], F32, tag="rs")
        nc.scalar.activation(out=rs[:], in_=rv[:], func=AF.Sqrt)
        asc = bn.tile([H, 1], F32, tag="asc")
        nc.vector.tensor_tensor(out=asc[:], in0=rs[:], in1=gam_sb[:], op=AT.mult)
        mb = bn.tile([H, 1], F32, tag="mb")
        nc.vector.tensor_tensor(out=mb[:], in0=mean[:], in1=asc[:], op=AT.mult)
        bsh = bn.tile([H, 1], F32, tag="bsh")
        nc.vector.tensor_tensor(out=bsh[:], in0=bet_sb[:], in1=mb[:], op=AT.subtract)
        OG = 8
        for n0 in range(0, NOW, OG):
            gn = min(OG, NOW - n0)
            yt8 = bn.tile([H, OG * 128], F32, tag="yt8")
            nc.vector.tensor_scalar(
                out=yt8[:, : gn * 128], in0=node[:, n0 * 128 : (n0 + gn) * 128],
                scalar1=asc[:], scalar2=bsh[:], op0=AT.mult, op1=AT.add)
            pt8 = bnp.tile([128, OG * H], F32, tag="pt8")
            for gi in range(gn):
                nc.tensor.transpose(out=pt8[:, gi * H : (gi + 1) * H],
                                    in_=yt8[:, gi * 128 : (gi + 1) * 128],
                                    identity=ident_sb[:])
            stg = bn.tile([128, OG * H], BF16, tag="stg")
            nc.scalar.activation(out=stg[:, : gn * H], in_=pt8[:, : gn * H],
                                 func=AF.Copy)
            eng = nc.scalar if (n0 // OG) % 2 else nc.sync
            eng.dma_start(out=out_ap[:, n0 * H : (n0 + gn) * H],
                          in_=stg[:, : gn * H])

    stack.close()


def make_nc(cfg):
    mu, mc = cfg["uu"], cfg["cu"]
    nc = bacc.Bacc(
        "TRN2",
        target_bir_lowering=False,
        debug=False,
        enable_asserts=False,
        num_devices=NCORES,
    )
    io = {}
    specs = [
        ("stream_u", (128, mu["NBLK"] * ED), BF16),
        ("stream_c", (128, mc["NBLK"] * DC), BF16),
        ("uloc", (ED, LP), BF16),
        ("iota8", (128, CHUNK * WIN), BF16),
        ("ident", (128, 128), F32),
        ("wg", (NH, ED, H), BF16),
        ("wr", (NH, ED, H), BF16),
        ("wl", (NH, DC, H), BF16),
        ("gcn_b", (NH, H), F32),
        ("sage_l_b", (NH, H), F32),
        ("bn_gamma", (H,), F32),
        ("bn_beta", (H,), F32),
        ("colw_u", (128, mu["NBLK"]), BF16),
        ("wdeg", (128, mu["NBLK"]), BF16),
        ("cdeg", (128, mu["NBLK"]), BF16),
        ("colw_c", (128, mc["NBLK"]), BF16),
        ("wcnt", (128, mc["NBLK"]), BF16),
    ]
    for name, shape, dt in specs:
        io[name] = nc.dram_tensor(name, list(shape), dt, kind="ExternalInput").ap()
    # output: [128, NOW*H] bf16, partition-contiguous; host relayouts
    out_ap = nc.dram_tensor("out_shard", [128, NOW * H], BF16,
                            kind="ExternalOutput").ap()
    with tile.TileContext(nc) as tc:
        build(nc, tc, io, out_ap, cfg)
    nc.compile()
    return nc


def kernel(**inputs):
    percore, cfg = host_prep(inputs)
    nc = make_nc(cfg)
    res = bass_utils.run_bass_kernel_spmd(nc, percore, core_ids=list(range(NCORES)))
    outs = []
    for k in range(NCORES):
        o = np.asarray(res.results[k]["out_shard"])      # [128, NOW*H] bf16
        o = o.reshape(128, NOW, H).transpose(1, 0, 2).reshape(LP, H)
        outs.append(o[:L])
    return np.concatenate(outs, axis=0).astype(np.float32)
